# revision 16
# baseline (speedup 1.0000x reference)
"""Trainium2 Bass kernel for nn_AttentionLayer (attention pooling over time).

Math (per sample b):
    logits[t] = u . tanh(X[b] @ W)[t]     # (T,)
    att       = softmax_t(logits)
    out[b]    = sum_t att[t] * X[b, t, :] # (D,)

Strategy:
  - Data-parallel over batch across 8 NeuronCores (B=64 -> 8 samples/core).
  - tanh bounds |logit| <= sum|u| < 5, so softmax needs NO max subtraction:
    p[t] = exp(logit[t]) is safe in fp32.  That removes the softmax barrier
    and allows a single streaming pass over X with PSUM accumulation of both
    sum_t p[t]*x[t] and sum_t p[t]; one divide per sample at the end.
  - The X@W matmul contracts over d, so it needs X^T (d on partitions); the
    weighted sum contracts over t, so it needs X natural (t on partitions).
    The host pre-casts X to bf16 in BOTH layouts; total HBM bytes per core
    equal one fp32 pass of X, and no on-chip transpose is needed.
  - All matmuls bf16 (1 cycle/row on PE) with fp32 PSUM accumulation.
  - DMA is issued as one 2 MiB slab per sample per layout.  The natural
    layout maps t-rows p*NS+s to partition p so each partition is one
    16 KiB contiguous run; the transposed layout is stored by the host in
    the matching permuted t-order j = s*128 + p (t = NS*p + s), so the
    logits produced from X^T columns line up partition-for-partition with
    the natural-layout subtiles used by the weighted sum.
  - The per-supertile chain xw -> tanh -> logits -> exp -> weighted-sum is
    software-pipelined two supertiles deep so PE never sits behind ACT.
"""

import numpy as np
import ml_dtypes

B, T, D, CTX = 64, 4096, 256, 100
NCORES = 8
BPC = B // NCORES          # samples per core
CP = 128                   # context dim padded to 128 (W/u zero-padded)
TSUP = 512                 # t-rows per supertile (one PSUM bank of xw)
BF16 = ml_dtypes.bfloat16

_NC_CACHE: dict = {}


def build_nc(bpc=BPC, t_total=T):
    """Build (and cache) the Bass graph for one core's shard."""
    key = (bpc, t_total)
    if key in _NC_CACHE:
        return _NC_CACHE[key]

    from contextlib import ExitStack
    import concourse.bass as bass
    import concourse.tile as tile
    from concourse import bacc, mybir

    nsup = t_total // TSUP     # supertiles per sample
    t_half = t_total // 2      # DMA slab = half a sample per layout
    nsup_h = nsup // 2         # supertiles per half-slab
    ns_h = t_half // 128       # t-rows per partition in one natural slab

    nc = bacc.Bacc("TRN2", target_bir_lowering=False, debug=False)
    x = nc.declare_dram_parameter("x", [bpc, t_total, D], mybir.dt.bfloat16,
                                  isOutput=False)
    xt = nc.declare_dram_parameter("xt", [bpc, 2, D, t_half],
                                   mybir.dt.bfloat16, isOutput=False)
    w = nc.declare_dram_parameter("w", [D, CP], mybir.dt.bfloat16,
                                  isOutput=False)
    u = nc.declare_dram_parameter("u", [CP, 1], mybir.dt.bfloat16,
                                  isOutput=False)
    out = nc.declare_dram_parameter("out", [bpc, D], mybir.dt.float32,
                                    isOutput=True)

    FP32 = mybir.dt.float32
    BF = mybir.dt.bfloat16
    PSUM = bass.MemorySpace.PSUM
    AF = mybir.ActivationFunctionType

    with tile.TileContext(nc) as tc:
        with ExitStack() as ctx:
            const = ctx.enter_context(tc.tile_pool(name="const", bufs=1))
            xpool = ctx.enter_context(tc.tile_pool(name="x", bufs=6))
            xtpool = ctx.enter_context(tc.tile_pool(name="xt", bufs=6))
            thpool = ctx.enter_context(tc.tile_pool(name="th", bufs=4))
            ppool = ctx.enter_context(tc.tile_pool(name="p", bufs=4))
            fin = ctx.enter_context(tc.tile_pool(name="fin", bufs=4))
            xwps = ctx.enter_context(tc.tile_pool(name="xwps", bufs=3, space=PSUM))
            paps = ctx.enter_context(tc.tile_pool(name="paps", bufs=2, space=PSUM))
            oaps = ctx.enter_context(tc.tile_pool(name="oaps", bufs=2, space=PSUM))
            s1ps = ctx.enter_context(tc.tile_pool(name="s1ps", bufs=1, space=PSUM))

            # Constants: W chunked [d', c_chunk, m], u, ones column.
            w_sb = const.tile([128, 2, CP], BF, tag="w")
            nc.gpsimd.dma_start(w_sb[:], w.rearrange("(c p) m -> p c m", p=128))
            u_sb = const.tile([CP, 1], BF, tag="u")
            nc.gpsimd.dma_start(u_sb[:], u[:, :])
            onesf_sb = const.tile([128, 1], FP32, tag="onesf")
            nc.vector.memset(onesf_sb[:], 1.0)

            # State per sample, filled as the pipeline flows.
            xn = [None] * bpc
            xtt = [None] * bpc
            oacc = [None] * bpc
            scols = [None] * bpc
            th = {}
            pacc = {}
            p_sb = {}

            def supt(g):
                return divmod(g, nsup)  # -> (sample, supertile-in-sample)

            # One continuous software pipeline over ALL supertiles of all
            # samples: stage A/B at g, C/D at g-1, E at g-2.  Within each
            # iteration, PE work is emitted ready-first (E, C, then A),
            # because PE executes in order and A may still be waiting on
            # its DMA slab.
            ntot = bpc * nsup
            for g in range(ntot + 2):
                # --- A/B: xw + tanh for supertile g (+ slab DMAs) ---
                if g < ntot:
                    b, st = supt(g)
                    if st == 0:
                        # Two 1 MiB DMA slabs per layout per sample
                        # (t-halves), xt first since it heads the compute
                        # pipeline.  Runs per partition stay 4/8 KiB.
                        xtt[b] = [None, None]
                        xn[b] = [None, None]
                        for h in range(2):
                            xtt[b][h] = xtpool.tile(
                                [128, 2, t_half], BF, tag="xtt",
                                name=f"xtt{b}_{h}")
                            nc.sync.dma_start(
                                xtt[b][h][:],
                                xt[b, h].rearrange("(c p) t -> p c t", p=128))
                            xn[b][h] = xpool.tile(
                                [128, ns_h, D], BF, tag="xn",
                                name=f"xn{b}_{h}")
                            nc.sync.dma_start(
                                xn[b][h][:],
                                x[b, h * t_half:(h + 1) * t_half, :].rearrange(
                                    "(p s) d -> p s d", p=128))
                        oacc[b] = oaps.tile([1, D], FP32, tag="oacc",
                                            name=f"oacc{b}")
                        scols[b] = ppool.tile([128, nsup], FP32, tag="scols",
                                              name=f"scols{b}")

                    h = st // nsup_h
                    j0 = (st % nsup_h) * TSUP
                    xwp = xwps.tile([128, TSUP], FP32, tag="xw",
                                    name=f"xw{g}")
                    for c in range(2):
                        nc.tensor.matmul(xwp[:], w_sb[:, c, :],
                                         xtt[b][h][:, c, j0:j0 + TSUP],
                                         start=(c == 0), stop=(c == 1))
                    th[g] = thpool.tile([128, TSUP], BF, tag="th",
                                        name=f"th{g}")
                    nc.scalar.activation(th[g][:], xwp[:], AF.Tanh)

                # --- C/D: logits + exp for supertile g-1 ---
                if 1 <= g <= ntot:
                    b, st = supt(g - 1)
                    pacc[g - 1] = paps.tile([128, 4], FP32, tag="pacc",
                                            name=f"pacc{g - 1}")
                    for s in range(4):
                        nc.tensor.matmul(pacc[g - 1][:, s:s + 1],
                                         th[g - 1][:, s * 128:(s + 1) * 128],
                                         u_sb[:],
                                         start=(s == 0), stop=(s == 3))
                    p_sb[g - 1] = ppool.tile([128, 4], BF, tag="p",
                                             name=f"p{g - 1}")
                    nc.scalar.activation(p_sb[g - 1][:], pacc[g - 1][:],
                                         AF.Exp,
                                         accum_out=scols[b][:, st:st + 1])

                # --- E: weighted-sum matmuls for supertile g-2 ---
                if g >= 2:
                    b, st = supt(g - 2)
                    for s in range(4):
                        sg = 4 * st + s
                        h2, sl = sg // ns_h, sg % ns_h
                        nc.tensor.matmul(oacc[b][:],
                                         p_sb[g - 2][:, s:s + 1],
                                         xn[b][h2][:, sl, :],
                                         start=(sg == 0),
                                         stop=(sg == 4 * nsup - 1))
                    if st == nsup - 1:
                        # Finalize sample b: out_row = oacc / sum_t p
                        s1v = fin.tile([128, 1], FP32, tag="s1v",
                                       name=f"s1v{b}")
                        nc.vector.reduce_sum(s1v[:], scols[b][:],
                                             axis=mybir.AxisListType.X)
                        s1 = s1ps.tile([1, 1], FP32, tag="s1",
                                       name=f"s1_{b}")
                        nc.tensor.matmul(s1[:], onesf_sb[:], s1v[:])
                        rinv = fin.tile([1, 1], FP32, tag="rinv",
                                        name=f"rinv{b}")
                        nc.vector.reciprocal(rinv[:], s1[:])
                        osb = fin.tile([1, D], FP32, tag="osb",
                                       name=f"osb{b}")
                        nc.vector.tensor_scalar_mul(osb[:], oacc[b][:],
                                                    rinv[:])
                        nc.gpsimd.dma_start(out[b:b + 1, :], osb[:])

    nc.compile()
    _NC_CACHE[key] = nc
    return nc


def make_in_maps(X, W, u, ncores=NCORES):
    """Shard + cast the full inputs for the cores.

    xt is stored t-permuted: column j = s*128 + p holds X[t = NS*p + s, :],
    matching the natural slab's partition layout (see build_nc docstring).
    """
    Xf = np.asarray(X)
    bpc = Xf.shape[0] // ncores
    t_total = Xf.shape[1]
    ns = t_total // 128
    Wp = np.zeros((D, CP), dtype=BF16)
    Wp[:, :CTX] = np.asarray(W).astype(BF16)
    up = np.zeros((CP, 1), dtype=BF16)
    up[:CTX, :] = np.asarray(u).astype(BF16)
    X16 = Xf.astype(BF16)
    in_maps = []
    for i in range(ncores):
        xs = np.ascontiguousarray(X16[i * bpc:(i + 1) * bpc])
        # per half: [b, h, 128p, s, d] -> [b, h, d, s, p]; j = s*128 + p
        ns_h = ns // 2
        xts = np.ascontiguousarray(
            xs.reshape(bpc, 2, 128, ns_h, D).transpose(0, 1, 4, 3, 2)
        ).reshape(bpc, 2, D, t_total // 2)
        in_maps.append({"x": xs, "xt": xts, "w": Wp, "u": up})
    return in_maps


# test.py sets _PROFILE=True to capture neuron-profile exec time here.
_PROFILE = False
LAST_RESULT = None


def kernel(X, W, u):
    global LAST_RESULT
    from concourse.bass_utils import run_bass_kernel_spmd

    nc = build_nc()
    in_maps = make_in_maps(X, W, u)
    res = run_bass_kernel_spmd(nc, in_maps, core_ids=list(range(NCORES)),
                               trace=_PROFILE)
    LAST_RESULT = res
    outs = [np.asarray(res.results[i]["out"], dtype=np.float32)
            for i in range(NCORES)]
    return np.concatenate(outs, axis=0)


# revision 18
# speedup vs baseline: 1.1162x; 1.1162x over previous
"""Trainium2 Bass kernel for nn_AttentionLayer (attention pooling over time).

Math (per sample b):
    logits[t] = u . tanh(X[b] @ W)[t]     # (T,)
    att       = softmax_t(logits)
    out[b]    = sum_t att[t] * X[b, t, :] # (D,)

Strategy:
  - Data-parallel over batch across 8 NeuronCores (B=64 -> 8 samples/core).
  - tanh bounds |logit| <= sum|u| < 5, so softmax needs NO max subtraction:
    p[t] = exp(logit[t]) is safe in fp32.  That removes the softmax barrier
    and allows a single streaming pass over X with PSUM accumulation of both
    sum_t p[t]*x[t] and sum_t p[t]; one divide per sample at the end.
  - The X@W matmul contracts over d, so it needs X^T (d on partitions); the
    weighted sum contracts over t, so it needs X natural (t on partitions).
    The host pre-casts X to bf16 in BOTH layouts; total HBM bytes per core
    equal one fp32 pass of X, and no on-chip transpose is needed.
  - All matmuls bf16 (1 cycle/row on PE) with fp32 PSUM accumulation.
  - DMA is issued as one 2 MiB slab per sample per layout.  The natural
    layout maps t-rows p*NS+s to partition p so each partition is one
    16 KiB contiguous run; the transposed layout is stored by the host in
    the matching permuted t-order j = s*128 + p (t = NS*p + s), so the
    logits produced from X^T columns line up partition-for-partition with
    the natural-layout subtiles used by the weighted sum.
  - The per-supertile chain xw -> tanh -> logits -> exp -> weighted-sum is
    software-pipelined two supertiles deep so PE never sits behind ACT.
"""

import numpy as np
import ml_dtypes

B, T, D, CTX = 64, 4096, 256, 100
NCORES = 8
BPC = B // NCORES          # samples per core
CP = 128                   # context dim padded to 128 (W/u zero-padded)
TSUP = 512                 # t-rows per supertile (one PSUM bank of xw)
BF16 = ml_dtypes.bfloat16

_NC_CACHE: dict = {}


def build_nc(bpc=BPC, t_total=T):
    """Build (and cache) the Bass graph for one core's shard."""
    key = (bpc, t_total)
    if key in _NC_CACHE:
        return _NC_CACHE[key]

    from contextlib import ExitStack
    import concourse.bass as bass
    import concourse.tile as tile
    from concourse import bacc, mybir

    nsup = t_total // TSUP     # supertiles per sample
    t_half = t_total // 2      # DMA slab = half a sample per layout
    nsup_h = nsup // 2         # supertiles per half-slab
    ns_h = t_half // 128       # t-rows per partition in one natural slab

    nc = bacc.Bacc("TRN2", target_bir_lowering=False, debug=False)
    x = nc.declare_dram_parameter("x", [bpc, t_total, D], mybir.dt.bfloat16,
                                  isOutput=False)
    xt = nc.declare_dram_parameter("xt", [bpc, 2, D, t_half],
                                   mybir.dt.float8e4, isOutput=False)
    w = nc.declare_dram_parameter("w", [D, CP], mybir.dt.float8e4,
                                  isOutput=False)
    u = nc.declare_dram_parameter("u", [CP, 1], mybir.dt.bfloat16,
                                  isOutput=False)
    out = nc.declare_dram_parameter("out", [bpc, D], mybir.dt.float32,
                                    isOutput=True)

    FP32 = mybir.dt.float32
    BF = mybir.dt.bfloat16
    F8 = mybir.dt.float8e4
    PSUM = bass.MemorySpace.PSUM
    AF = mybir.ActivationFunctionType

    with tile.TileContext(nc) as tc:
        with ExitStack() as ctx:
            const = ctx.enter_context(tc.tile_pool(name="const", bufs=1))
            xpool = ctx.enter_context(tc.tile_pool(name="x", bufs=6))
            xtpool = ctx.enter_context(tc.tile_pool(name="xt", bufs=6))
            thpool = ctx.enter_context(tc.tile_pool(name="th", bufs=4))
            ppool = ctx.enter_context(tc.tile_pool(name="p", bufs=4))
            fin = ctx.enter_context(tc.tile_pool(name="fin", bufs=4))
            xwps = ctx.enter_context(tc.tile_pool(name="xwps", bufs=3, space=PSUM))
            paps = ctx.enter_context(tc.tile_pool(name="paps", bufs=2, space=PSUM))
            oaps = ctx.enter_context(tc.tile_pool(name="oaps", bufs=2, space=PSUM))
            s1ps = ctx.enter_context(tc.tile_pool(name="s1ps", bufs=1, space=PSUM))

            # Constants: W chunked [d', c_chunk, m], u, ones column.
            w_sb = const.tile([128, 2, CP], F8, tag="w")
            nc.gpsimd.dma_start(w_sb[:], w.rearrange("(c p) m -> p c m", p=128))
            u_sb = const.tile([CP, 1], BF, tag="u")
            nc.gpsimd.dma_start(u_sb[:], u[:, :])
            onesf_sb = const.tile([128, 1], FP32, tag="onesf")
            nc.vector.memset(onesf_sb[:], 1.0)

            # State per sample, filled as the pipeline flows.
            xn = [None] * bpc
            xtt = [None] * bpc
            oacc = [None] * bpc
            scols = [None] * bpc
            th = {}
            pacc = {}
            p_sb = {}

            def supt(g):
                return divmod(g, nsup)  # -> (sample, supertile-in-sample)

            # One continuous software pipeline over ALL supertiles of all
            # samples: stage A/B at g, C/D at g-1, E at g-2.  Within each
            # iteration, PE work is emitted ready-first (E, C, then A),
            # because PE executes in order and A may still be waiting on
            # its DMA slab.
            ntot = bpc * nsup
            for g in range(ntot + 2):
                # --- E: weighted-sum matmuls for supertile g-2 ---
                if g >= 2:
                    b, st = supt(g - 2)
                    for s in range(4):
                        sg = 4 * st + s
                        h2, sl = sg // ns_h, sg % ns_h
                        nc.tensor.matmul(oacc[b][:],
                                         p_sb[g - 2][:, s:s + 1],
                                         xn[b][h2][:, sl, :],
                                         start=(sg == 0),
                                         stop=(sg == 4 * nsup - 1))
                    if st == nsup - 1:
                        # Finalize sample b: out_row = oacc / sum_t p
                        s1v = fin.tile([128, 1], FP32, tag="s1v",
                                       name=f"s1v{b}")
                        nc.vector.reduce_sum(s1v[:], scols[b][:],
                                             axis=mybir.AxisListType.X)
                        s1 = s1ps.tile([1, 1], FP32, tag="s1",
                                       name=f"s1_{b}")
                        nc.tensor.matmul(s1[:], onesf_sb[:], s1v[:])
                        rinv = fin.tile([1, 1], FP32, tag="rinv",
                                        name=f"rinv{b}")
                        nc.vector.reciprocal(rinv[:], s1[:])
                        osb = fin.tile([1, D], FP32, tag="osb",
                                       name=f"osb{b}")
                        nc.vector.tensor_scalar_mul(osb[:], oacc[b][:],
                                                    rinv[:])
                        nc.gpsimd.dma_start(out[b:b + 1, :], osb[:])

                # --- C/D: logits + exp for supertile g-1 ---
                if 1 <= g <= ntot:
                    b, st = supt(g - 1)
                    pacc[g - 1] = paps.tile([128, 4], FP32, tag="pacc",
                                            name=f"pacc{g - 1}")
                    for s in range(4):
                        nc.tensor.matmul(pacc[g - 1][:, s:s + 1],
                                         th[g - 1][:, s * 128:(s + 1) * 128],
                                         u_sb[:],
                                         start=(s == 0), stop=(s == 3))
                    p_sb[g - 1] = ppool.tile([128, 4], BF, tag="p",
                                             name=f"p{g - 1}")
                    nc.scalar.activation(p_sb[g - 1][:], pacc[g - 1][:],
                                         AF.Exp,
                                         accum_out=scols[b][:, st:st + 1])

                # --- A/B: xw + tanh for supertile g (+ slab DMAs) ---
                if g < ntot:
                    b, st = supt(g)
                    if st == 0:
                        # Two 1 MiB DMA slabs per layout per sample
                        # (t-halves), xt first since it heads the compute
                        # pipeline.  Runs per partition stay 4/8 KiB.
                        xtt[b] = [None, None]
                        xn[b] = [None, None]
                        for h in range(2):
                            xtt[b][h] = xtpool.tile(
                                [128, 2, t_half], F8, tag="xtt",
                                name=f"xtt{b}_{h}")
                            nc.sync.dma_start(
                                xtt[b][h][:],
                                xt[b, h].rearrange("(c p) t -> p c t", p=128))
                            xn[b][h] = xpool.tile(
                                [128, ns_h, D], BF, tag="xn",
                                name=f"xn{b}_{h}")
                            nc.sync.dma_start(
                                xn[b][h][:],
                                x[b, h * t_half:(h + 1) * t_half, :].rearrange(
                                    "(p s) d -> p s d", p=128))
                        oacc[b] = oaps.tile([1, D], FP32, tag="oacc",
                                            name=f"oacc{b}")
                        scols[b] = ppool.tile([128, nsup], FP32, tag="scols",
                                              name=f"scols{b}")

                    h = st // nsup_h
                    j0 = (st % nsup_h) * TSUP
                    xwp = xwps.tile([128, TSUP], FP32, tag="xw",
                                    name=f"xw{g}")
                    nc.tensor.matmul(xwp[:], w_sb[:, :, :],
                                     xtt[b][h][:, :, j0:j0 + TSUP],
                                     perf_mode=mybir.MatmulPerfMode.DoubleRow)
                    th[g] = thpool.tile([128, TSUP], BF, tag="th",
                                        name=f"th{g}")
                    nc.scalar.activation(th[g][:], xwp[:], AF.Tanh)

    nc.compile()
    _NC_CACHE[key] = nc
    return nc


def make_in_maps(X, W, u, ncores=NCORES):
    """Shard + cast the full inputs for the cores.

    xt is stored t-permuted: column j = s*128 + p holds X[t = NS*p + s, :],
    matching the natural slab's partition layout (see build_nc docstring).
    """
    Xf = np.asarray(X)
    bpc = Xf.shape[0] // ncores
    t_total = Xf.shape[1]
    ns = t_total // 128
    F8 = ml_dtypes.float8_e4m3
    Wp = np.zeros((D, CP), dtype=F8)
    Wp[:, :CTX] = np.asarray(W).astype(F8)
    up = np.zeros((CP, 1), dtype=BF16)
    up[:CTX, :] = np.asarray(u).astype(BF16)
    X16 = Xf.astype(BF16)
    in_maps = []
    for i in range(ncores):
        xs = np.ascontiguousarray(X16[i * bpc:(i + 1) * bpc])
        # per half: [b, h, 128p, s, d] -> [b, h, d, s, p]; j = s*128 + p
        # fp8 for the logits path only (cast from full-precision X)
        ns_h = ns // 2
        xs8 = Xf[i * bpc:(i + 1) * bpc].astype(F8)
        xts = np.ascontiguousarray(
            xs8.reshape(bpc, 2, 128, ns_h, D).transpose(0, 1, 4, 3, 2)
        ).reshape(bpc, 2, D, t_total // 2)
        in_maps.append({"x": xs, "xt": xts, "w": Wp, "u": up})
    return in_maps


# test.py sets _PROFILE=True to capture neuron-profile exec time here.
_PROFILE = False
LAST_RESULT = None


def kernel(X, W, u):
    global LAST_RESULT
    from concourse.bass_utils import run_bass_kernel_spmd

    nc = build_nc()
    in_maps = make_in_maps(X, W, u)
    res = run_bass_kernel_spmd(nc, in_maps, core_ids=list(range(NCORES)),
                               trace=_PROFILE)
    LAST_RESULT = res
    outs = [np.asarray(res.results[i]["out"], dtype=np.float32)
            for i in range(NCORES)]
    return np.concatenate(outs, axis=0)
